# revision 18
# baseline (speedup 1.0000x reference)
"""CLCE loss kernel for Trainium2 (8 NeuronCores, SPMD) — symmetric-triangle.

Loss = 0.5 * cl + 0.5 * ce where
  cl_i = log(exp(slot0_i) + (T_i - P_i) + (2N-2 - num_neg_i)) - slot0_i
  T_i  = sum_j exp((xn_i . xn_j + 1) * 0.25)      <- O(N^2 D), on device
  P_i, slot0_i, ce assembled on host as in the validated baseline.

exp(sim) is symmetric, so only a triangle cover of the 8x8 grid of
512x512 blocks is computed (4.5 block-units per core instead of 8):
  core k: diag (k,k)  [row-sums only]
          (k, k+d) for d=1..3  [row-sums + column-sums]
          half of the d=4 block: rows chunk p=k%4 (m01 if k<4 else m23)
          x cols chunk p+4  [row-sums + column-sums]
Row sums come free from the Scalar-engine exp accumulator; column sums
are DVE adds of the bf16 exp tiles over the m-tiles followed by a
ones-weight matmul (contraction over the 128 partitions).  Host adds the
per-block RS/CS pieces into T.  Matmul work: 72 DR-fp8 matmuls of 512
cols vs 128 in the full-row version.

Pipeline: phase 1 k-chases the arriving (A[k], B2[k]) DMA pairs with 6
matmuls per arrival (b1 m0-3 + b2 m0-1, 6 psum banks) so the PE never
idles on the DMA warm-up; phase 2 runs the remaining blocks m-outer.
All bulk input DMAs ride the sync HWDGE queue back-to-back in exact
consumption order (a deep queue backlog is what keeps the DMA engines
saturated); y_pred rides the scalar queue as bf16.
"""

import os
from contextlib import ExitStack

import numpy as np

import concourse.bass as bass
import concourse.tile as tile
from concourse import bacc, mybir
from concourse.bass_utils import run_bass_kernel_spmd

N, D, C = 4096, 1024, 512
TAU = 0.5
LAMBD = 0.5
NCORES = 8
BLK = 512                  # chunk width (rows per core block)
P = 128                    # partitions
KT = D // 256              # 4 DoubleRow contraction super-tiles
MT = BLK // P              # 4 m-tiles per full block
S8 = 16.0                  # fp8 pre-scale for the embeddings
AW = 2 * BLK               # A region: c0 (W + diag X) | c1
RW = 2 * BLK + 256         # BR region: c3 | c4 | wh

_F32 = mybir.dt.float32
_BF16 = mybir.dt.bfloat16
_FP8 = mybir.dt.float8e4
_EXP = mybir.ActivationFunctionType.Exp
_DR = mybir.MatmulPerfMode.DoubleRow

# OUTSB column map
RS_B1, RS_B2, RS_B3, RS_B4, RS_B0, RS_CE = 0, 4, 8, 12, 14, 18
NRS = 22
# OUTCS slot map (512 cols each)
CS_B1, CS_B2, CS_B3, CS_B4 = 0, 1, 2, 3


def _build_kernel(tc, biga, bigb2, bigbr, yp, out_rs, out_cs):
    """Emit the per-core Tile kernel.

    biga:  [KT*P, 2*AW]  fp8  per k: [c0 | c1] (W + diag X + b1 X)
    bigb2: [KT*P, 2*BLK] fp8  per k: c2 (b2 X)
    bigbr: [KT*P, 2*RW]  fp8  per k: [c3 | c4 | wh]
    yp:    [P, MT*C]     bf16 this core's y_pred block, partition-major
    out_rs: [P, 22]      f32  18 sim row-sum cols + 4 CE row-sums
    out_cs: [1, 2048]    f32  4 x 512 column-sum vectors
    """
    nc = tc.nc
    act_scale = 0.5 * TAU / (S8 * S8)
    with ExitStack() as ctx:
        pers = ctx.enter_context(tc.tile_pool(name="pers", bufs=1))
        epool = ctx.enter_context(tc.tile_pool(name="epool", bufs=3))
        cepool = ctx.enter_context(tc.tile_pool(name="cepool", bufs=2))
        apool = ctx.enter_context(tc.tile_pool(name="apool", bufs=4))
        psum = ctx.enter_context(
            tc.tile_pool(name="psum", bufs=6, space=bass.MemorySpace.PSUM)
        )
        cpsum = ctx.enter_context(
            tc.tile_pool(name="cpsum", bufs=2, space=bass.MemorySpace.PSUM)
        )

        A = [pers.tile([P, 2, AW], _FP8, name=f"ba{k}", tag=f"ba{k}")
             for k in range(KT)]
        B2 = [pers.tile([P, 2, BLK], _FP8, name=f"b2{k}", tag=f"b2{k}")
              for k in range(KT)]
        BR = [pers.tile([P, 2, RW], _FP8, name=f"br{k}", tag=f"br{k}")
              for k in range(KT)]
        YPB = pers.tile([P, MT * C], _BF16)
        OUTSB = pers.tile([P, NRS], _F32)
        OUTCS = pers.tile([1, 4 * BLK], _F32)
        bias_s = pers.tile([P, 1], _F32)
        bias_z = pers.tile([P, 1], _F32)
        warm = pers.tile([P, 1], _F32)
        ZW = pers.tile([P, 512], _BF16)
        ONES = pers.tile([P, P], _BF16)

        # input DMAs first: sync HWDGE queue, exact consumption order
        a3 = biga.rearrange("r (i n) -> r i n", i=2)
        b23 = bigb2.rearrange("r (i n) -> r i n", i=2)
        br3 = bigbr.rearrange("r (i n) -> r i n", i=2)
        for k in range(KT):
            nc.sync.dma_start(A[k][:], a3[k * P:(k + 1) * P, :, :])
            nc.sync.dma_start(B2[k][:], b23[k * P:(k + 1) * P, :, :])
        for k in range(KT):
            nc.sync.dma_start(BR[k][:], br3[k * P:(k + 1) * P, :, :])

        # vector engine: warm-up operands (its preamble ends earliest and
        # it has no other early work)
        nc.vector.memset(ZW[:], 0.0)
        nc.vector.memset(ONES[:], 1.0)
        # gpsimd: activation biases
        nc.gpsimd.memset(bias_z[:], 0.0)
        nc.gpsimd.memset(bias_s[:], 0.5 * TAU)

        # scalar: y_pred DMA rides the scalar HWDGE queue; exp-table warm
        nc.scalar.dma_start(YPB[:], yp[:])
        nc.scalar.activation(warm[:], bias_z[:], _EXP, bias=bias_z[:], scale=1.0)

        # PE warm-up: dummy matmuls flip the HAM clock gate while the
        # first input DMAs are in flight
        wps = psum.tile([P, BLK], _F32, tag="ps")
        for _ in range(3):
            nc.tensor.matmul(wps[:], ZW[:, 0:P], ZW[:], start=True, stop=True)

        def act_rs(ps, col, vred=False):
            """exp + row sum (ScalarE accumulator, or VectorE reduce when
            the et tile must reach the CS add chain early)."""
            et = epool.tile([P, BLK], _BF16, tag="et")
            if vred:
                nc.scalar.activation(
                    et[:], ps[:], _EXP, bias=bias_s[:], scale=act_scale,
                )
                nc.vector.tensor_reduce(
                    OUTSB[:, col:col + 1], et[:],
                    mybir.AxisListType.X, mybir.AluOpType.add,
                )
            else:
                nc.scalar.activation(
                    et[:], ps[:], _EXP, bias=bias_s[:], scale=act_scale,
                    accum_out=OUTSB[:, col:col + 1],
                )
            return et

        def ones_mm(acc, slot):
            cps = cpsum.tile([P, BLK], _F32, tag="cps")
            nc.tensor.matmul(cps[:], ONES[:], acc[:], start=True, stop=True)
            nc.vector.tensor_copy(
                OUTCS[0:1, slot * BLK:(slot + 1) * BLK], cps[0:1, :]
            )

        def cs_adds(ets, n):
            """pairwise-tree DVE adds of the exp tiles -> one bf16 tile."""
            if n == 2:
                x = apool.tile([P, BLK], _BF16, tag="acc")
                nc.vector.tensor_add(x[:], ets[0][:], ets[1][:])
                return x
            x01 = apool.tile([P, BLK], _BF16, tag="acc")
            nc.vector.tensor_add(x01[:], ets[0][:], ets[1][:])
            x23 = apool.tile([P, BLK], _BF16, tag="acc")
            nc.vector.tensor_add(x23[:], ets[2][:], ets[3][:])
            xs = apool.tile([P, BLK], _BF16, tag="acc")
            nc.vector.tensor_add(xs[:], x01[:], x23[:])
            return xs

        def mm(ps, k, m, xsrc, xoff, wsrc=None, woff=0):
            nc.tensor.matmul(
                ps[:],
                (wsrc or A)[k][:, :, woff + m * P: woff + (m + 1) * P],
                xsrc[k][:, :, xoff: xoff + BLK],
                start=(k == 0),
                stop=(k == KT - 1),
                perf_mode=_DR,
            )

        # --- phase 1: k-chase the (A[k], B2[k]) arrivals ---
        ps1 = [psum.tile([P, BLK], _F32, tag="ps", name=f"ps1_{m}")
               for m in range(MT)]
        ps2 = [psum.tile([P, BLK], _F32, tag="ps", name=f"ps2_{m}")
               for m in range(2)]
        for k in range(KT):
            for m in range(MT):
                mm(ps1[m], k, m, A, BLK)
            for m in range(2):
                mm(ps2[m], k, m, B2, 0)
            if k < KT - 1:  # HAM keep-alive if the next pair is late
                nc.tensor.matmul(wps[:], ZW[:, 0:P], ZW[:], start=True, stop=True)

        ets1 = [act_rs(ps1[m], RS_B1 + m) for m in range(MT)]
        csa1 = cs_adds(ets1, 4)

        # CE while ScalarE has slack: R[p, t] = sum_c exp(y_pred)
        for t in range(2):
            etc = cepool.tile([P, 2 * C], _F32, tag="etce")
            nc.scalar.activation(
                etc[:], YPB[:, 2 * t * C:(2 * t + 2) * C], _EXP,
                bias=bias_z[:], scale=1.0,
            )
            for h in range(2):
                nc.vector.tensor_reduce(
                    OUTSB[:, RS_CE + 2 * t + h:RS_CE + 2 * t + h + 1],
                    etc[:, h * C:(h + 1) * C],
                    mybir.AxisListType.X, mybir.AluOpType.add,
                )

        # --- phase 2 ---
        # b2 m2/m3 (m-outer), ACTs for b2 m0..3
        ets2 = [None] * MT
        psb = []
        for m in (2, 3):
            ps = psum.tile([P, BLK], _F32, tag="ps")
            for k in range(KT):
                mm(ps, k, m, B2, 0)
            psb.append(ps)
        ets2[0] = act_rs(ps2[0], RS_B2 + 0)
        ets2[1] = act_rs(ps2[1], RS_B2 + 1)
        ets2[2] = act_rs(psb[0], RS_B2 + 2)
        ets2[3] = act_rs(psb[1], RS_B2 + 3)
        csa2 = cs_adds(ets2, 4)

        # b3 (X = c3 at BR offset 0); onesMM(b1) after m0, (b2) after m2
        ets3 = []
        for m in range(MT):
            ps = psum.tile([P, BLK], _F32, tag="ps")
            for k in range(KT):
                mm(ps, k, m, BR, 0)
            ets3.append(act_rs(ps, RS_B3 + m))
            if m == 0:
                ones_mm(csa1, CS_B1)
            if m == 2:
                ones_mm(csa2, CS_B2)
        csa3 = cs_adds(ets3, 4)

        # b4 (half: W = wh at BR 2*BLK, X = c4 at BR BLK)
        ets4 = []
        for m in range(2):
            ps = psum.tile([P, BLK], _F32, tag="ps")
            for k in range(KT):
                mm(ps, k, m, BR, BLK, wsrc=BR, woff=2 * BLK)
            ets4.append(act_rs(ps, RS_B4 + m, vred=True))
        csa4 = cs_adds(ets4, 2)

        # b0 (diag, X = c0 at A offset 0): RS only, short tail
        for m in range(MT):
            ps = psum.tile([P, BLK], _F32, tag="ps")
            for k in range(KT):
                mm(ps, k, m, A, 0)
            act_rs(ps, RS_B0 + m)
            if m == 0:
                ones_mm(csa3, CS_B3)
            if m == 1:
                ones_mm(csa4, CS_B4)
            if m == 2:
                nc.gpsimd.dma_start(out_cs[:], OUTCS[:])

        nc.scalar.dma_start(out_rs[:], OUTSB[:])


_NC_CACHE = None


def _get_nc():
    global _NC_CACHE
    if _NC_CACHE is None:
        nc = bacc.Bacc(
            "TRN2", target_bir_lowering=False, debug=False,
            enable_asserts=False, num_devices=NCORES,
        )
        biga_d = nc.dram_tensor("biga", [KT * P, 2 * AW], _FP8, kind="ExternalInput")
        bigb2_d = nc.dram_tensor("bigb2", [KT * P, 2 * BLK], _FP8, kind="ExternalInput")
        bigbr_d = nc.dram_tensor("bigbr", [KT * P, 2 * RW], _FP8, kind="ExternalInput")
        yp_d = nc.dram_tensor("yp", [P, MT * C], _BF16, kind="ExternalInput")
        out_rs_d = nc.dram_tensor("out_rs", [P, NRS], _F32, kind="ExternalOutput")
        out_cs_d = nc.dram_tensor("out_cs", [1, 4 * BLK], _F32, kind="ExternalOutput")
        with tile.TileContext(nc) as tc:
            _build_kernel(
                tc, biga_d.ap(), bigb2_d.ap(), bigbr_d.ap(), yp_d.ap(),
                out_rs_d.ap(), out_cs_d.ap(),
            )
        nc.compile()
        _NC_CACHE = nc
    return _NC_CACHE


def _pack_cols(zq, cols_list):
    """[D, *] fp8 col-chunks -> [KT*P, 2*W] with the DoubleRow pairing
    row kk*128+p, col i*W+n  <->  contraction index kk*256 + 128i + p."""
    cat = np.concatenate(cols_list, axis=1)  # [D, W]
    w = cat.shape[1]
    return np.ascontiguousarray(
        cat.reshape(KT, 2, P, w).transpose(0, 2, 1, 3).reshape(KT * P, 2 * w)
    )


def _run_device(xnT, y_pred, trace=False):
    """Run the SPMD kernel; returns (T[N], R[N]) f64 and the raw results."""
    fp8np = mybir.dt.np(_FP8)
    bf16np = mybir.dt.np(_BF16)
    zq = (xnT * S8).astype(np.float32).astype(fp8np)  # [D, N] fp8
    ch = lambda j: zq[:, (j % 8) * BLK:(j % 8) * BLK + BLK]
    in_maps = []
    for c in range(NCORES):
        blk = slice(c * BLK, (c + 1) * BLK)
        ypb = (
            np.ascontiguousarray(y_pred[blk])
            .reshape(MT, P, C).transpose(1, 0, 2).reshape(P, MT * C)
        )
        if c < 4:
            c4 = ch(c + 4)
            wh = ch(c)[:, 0:256]
        else:
            c4 = ch(c)
            wh = ch(c - 4)[:, 256:512]
        in_maps.append({
            "biga": _pack_cols(zq, [ch(c), ch(c + 1)]),
            "bigb2": _pack_cols(zq, [ch(c + 2)]),
            "bigbr": _pack_cols(zq, [ch(c + 3), c4, wh]),
            "yp": np.ascontiguousarray(ypb).astype(bf16np),
        })
    res = run_bass_kernel_spmd(
        _get_nc(), in_maps, core_ids=list(range(NCORES)), trace=trace,
    )
    T = np.zeros(N, np.float64)
    R = np.empty(N, np.float64)
    for c, r in enumerate(res.results):
        o = r["out_rs"].astype(np.float64)            # [128, 22]
        cs = r["out_cs"].astype(np.float64).reshape(4, BLK)
        for m in range(MT):
            rows = slice(c * BLK + m * P, c * BLK + (m + 1) * P)
            T[rows] += (o[:, RS_B1 + m] + o[:, RS_B2 + m]
                        + o[:, RS_B3 + m] + o[:, RS_B0 + m])
            R[rows] = o[:, RS_CE + m]
        # half-block row sums
        if c < 4:
            p0 = c * BLK
            T[p0:p0 + P] += o[:, RS_B4]
            T[p0 + P:p0 + 2 * P] += o[:, RS_B4 + 1]
        else:
            p0 = (c - 4) * BLK
            T[p0 + 2 * P:p0 + 3 * P] += o[:, RS_B4]
            T[p0 + 3 * P:p0 + 4 * P] += o[:, RS_B4 + 1]
        # column sums
        chs = lambda j: slice((j % 8) * BLK, (j % 8) * BLK + BLK)
        T[chs(c + 1)] += cs[CS_B1]
        T[chs(c + 2)] += cs[CS_B2]
        T[chs(c + 3)] += cs[CS_B3]
        T[chs(c + 4 if c < 4 else c)] += cs[CS_B4]
    return T, R, res


def kernel(layer_embeds, y_true, y_pred):
    x = np.asarray(layer_embeds, dtype=np.float32)
    yt = np.asarray(y_true).astype(np.int64)
    yp = np.asarray(y_pred, dtype=np.float32)

    # normalize rows (torch-style eps clip)
    norms = np.maximum(
        np.sqrt((x.astype(np.float64) ** 2).sum(1, keepdims=True)), 1e-8
    )
    xn = (x / norms).astype(np.float32)
    xnT = np.ascontiguousarray(xn.T)  # [D, N]

    trace = bool(int(os.environ.get("CLCE_TRACE", "0")))
    T, R, res = _run_device(xnT, yp, trace=trace)
    if trace:
        kernel.last_results = res

    # --- host-side small terms (O(N * class_size)) ---
    # P_ must match what the device summed for the same-class entries, i.e.
    # the fp8-quantized sim values, so quantize the same way here.
    fp8np = mybir.dt.np(_FP8)
    xq = (xn * S8).astype(fp8np).astype(np.float64) / S8  # device-visible xn
    counts = np.bincount(yt, minlength=C)
    P_ = np.zeros(N, np.float64)
    slot0 = np.zeros(N, np.float64)
    for cval in np.unique(yt):
        idx = np.where(yt == cval)[0]
        subq = xq[idx]
        sq = (subq @ subq.T + 1.0) * (0.5 * TAU)   # device-matching sim
        P_[idx] = np.exp(sq).sum(1)
        if len(idx) >= 2:
            # slot0 feeds the final formula directly -> use full precision
            sub = xn[idx].astype(np.float64)
            s = (sub @ sub.T + 1.0) * (0.5 * TAU)
            firstpos = np.where(np.arange(len(idx)) == 0, 1, 0)
            slot0[idx] = s[np.arange(len(idx)), firstpos]

    num_neg = N - counts[yt]
    S = T - P_
    Z = (2 * N - 2 - num_neg).astype(np.float64)
    cl = (np.log(np.exp(slot0) + S + Z) - slot0).mean()
    ce = (
        np.log(R) - yp[np.arange(N), yt].astype(np.float64)
    ).mean()
    loss = LAMBD * cl + (1.0 - LAMBD) * ce
    return np.asarray(loss, dtype=np.float32)


# revision 19
# speedup vs baseline: 1.1053x; 1.1053x over previous
"""CLCE loss kernel for Trainium2 (8 NeuronCores, SPMD) — symmetric-triangle.

Loss = 0.5 * cl + 0.5 * ce where
  cl_i = log(exp(slot0_i) + (T_i - P_i) + (2N-2 - num_neg_i)) - slot0_i
  T_i  = sum_j exp((xn_i . xn_j + 1) * 0.25)      <- O(N^2 D), on device
  P_i, slot0_i, ce assembled on host as in the validated baseline.

exp(sim) is symmetric, so only a triangle cover of the 8x8 grid of
512x512 blocks is computed (4.5 block-units per core instead of 8):
  core k: diag (k,k)  [row-sums only]
          (k, k+d) for d=1..3  [row-sums + column-sums]
          half of the d=4 block: rows chunk p=k%4 (m01 if k<4 else m23)
          x cols chunk p+4  [row-sums + column-sums]
Row sums come free from the Scalar-engine exp accumulator; column sums
are DVE adds of the bf16 exp tiles over the m-tiles followed by a
ones-weight matmul (contraction over the 128 partitions).  Host adds the
per-block RS/CS pieces into T.  Matmul work: 72 DR-fp8 matmuls of 512
cols vs 128 in the full-row version.

Pipeline: phase 1 k-chases the arriving (A[k], B2[k]) DMA pairs with 6
matmuls per arrival (b1 m0-3 + b2 m0-1, 6 psum banks) so the PE never
idles on the DMA warm-up; phase 2 runs the remaining blocks m-outer.
All bulk input DMAs ride the sync HWDGE queue back-to-back in exact
consumption order (a deep queue backlog is what keeps the DMA engines
saturated); y_pred rides the scalar queue as bf16.
"""

import os
from contextlib import ExitStack

import numpy as np

import concourse.bass as bass
import concourse.tile as tile
from concourse import bacc, mybir
from concourse.bass_utils import run_bass_kernel_spmd

N, D, C = 4096, 1024, 512
TAU = 0.5
LAMBD = 0.5
NCORES = 8
BLK = 512                  # chunk width (rows per core block)
P = 128                    # partitions
KT = D // 256              # 4 DoubleRow contraction super-tiles
MT = BLK // P              # 4 m-tiles per full block
S8 = 16.0                  # fp8 pre-scale for the embeddings
AW = 2 * BLK               # A region: c0 (W + diag X) | c1
RW = 2 * BLK + 256         # BR region: c3 | c4 | wh

_F32 = mybir.dt.float32
_BF16 = mybir.dt.bfloat16
_FP8 = mybir.dt.float8e4
_EXP = mybir.ActivationFunctionType.Exp
_DR = mybir.MatmulPerfMode.DoubleRow

# OUTSB column map
RS_B1, RS_B2, RS_B3, RS_B4, RS_B0, RS_CE = 0, 4, 8, 12, 14, 18
NRS = 22
# OUTCS slot map (512 cols each)
CS_B1, CS_B2, CS_B3, CS_B4 = 0, 1, 2, 3


def _build_kernel(tc, biga, bigb2, bigbr, yp, out_rs, out_cs):
    """Emit the per-core Tile kernel.

    biga:  [KT*P, 2*AW]  fp8  per k: [c0 | c1] (W + diag X + b1 X)
    bigb2: [KT*P, 2*BLK] fp8  per k: c2 (b2 X)
    bigbr: [KT*P, 2*RW]  fp8  per k: [c3 | c4 | wh]
    yp:    [P, MT*C]     bf16 this core's y_pred block, partition-major
    out_rs: [P, 22]      f32  18 sim row-sum cols + 4 CE row-sums
    out_cs: [1, 2048]    f32  4 x 512 column-sum vectors
    """
    nc = tc.nc
    act_scale = 0.5 * TAU / (S8 * S8)
    with ExitStack() as ctx:
        pers = ctx.enter_context(tc.tile_pool(name="pers", bufs=1))
        epool = ctx.enter_context(tc.tile_pool(name="epool", bufs=3))
        cepool = ctx.enter_context(tc.tile_pool(name="cepool", bufs=2))
        apool = ctx.enter_context(tc.tile_pool(name="apool", bufs=4))
        psum = ctx.enter_context(
            tc.tile_pool(name="psum", bufs=6, space=bass.MemorySpace.PSUM)
        )
        cpsum = ctx.enter_context(
            tc.tile_pool(name="cpsum", bufs=2, space=bass.MemorySpace.PSUM)
        )

        A = [pers.tile([P, 2, AW], _FP8, name=f"ba{k}", tag=f"ba{k}")
             for k in range(KT)]
        B2 = [pers.tile([P, 2, BLK], _FP8, name=f"b2{k}", tag=f"b2{k}")
              for k in range(KT)]
        BR = [pers.tile([P, 2, RW], _FP8, name=f"br{k}", tag=f"br{k}")
              for k in range(KT)]
        YPB = pers.tile([P, MT * C], _BF16)
        OUTSB = pers.tile([P, NRS], _F32)
        OUTCS = pers.tile([1, 4 * BLK], _F32)
        bias_s = pers.tile([P, 1], _F32)
        bias_z = pers.tile([P, 1], _F32)
        warm = pers.tile([P, 1], _F32)
        ZW = pers.tile([P, 512], _BF16)
        ONES = pers.tile([P, P], _BF16)

        # input DMAs first: sync HWDGE queue, exact consumption order
        a3 = biga.rearrange("r (i n) -> r i n", i=2)
        b23 = bigb2.rearrange("r (i n) -> r i n", i=2)
        br3 = bigbr.rearrange("r (i n) -> r i n", i=2)
        for k in range(KT):
            nc.sync.dma_start(A[k][:], a3[k * P:(k + 1) * P, :, :])
            nc.sync.dma_start(B2[k][:], b23[k * P:(k + 1) * P, :, :])
        for k in range(KT):
            nc.sync.dma_start(BR[k][:], br3[k * P:(k + 1) * P, :, :])

        # vector engine: warm-up operands (its preamble ends earliest and
        # it has no other early work)
        nc.vector.memset(ZW[:], 0.0)
        nc.vector.memset(ONES[:], 1.0)
        # gpsimd: activation biases
        nc.gpsimd.memset(bias_z[:], 0.0)
        nc.gpsimd.memset(bias_s[:], 0.5 * TAU)

        # scalar: y_pred DMA rides the scalar HWDGE queue; exp-table warm
        nc.scalar.dma_start(YPB[:], yp[:])
        nc.scalar.activation(warm[:], bias_z[:], _EXP, bias=bias_z[:], scale=1.0)

        # PE warm-up: dummy matmuls flip the HAM clock gate while the
        # first input DMAs are in flight
        wps = psum.tile([P, BLK], _F32, tag="ps")
        for _ in range(3):
            nc.tensor.matmul(wps[:], ZW[:, 0:P], ZW[:], start=True, stop=True)

        def act_rs(ps, col):
            """exp + row-sum accumulate; returns the bf16 exp tile."""
            et = epool.tile([P, BLK], _BF16, tag="et")
            nc.scalar.activation(
                et[:], ps[:], _EXP, bias=bias_s[:], scale=act_scale,
                accum_out=OUTSB[:, col:col + 1],
            )
            return et

        def ones_mm(acc, slot):
            cps = cpsum.tile([P, BLK], _F32, tag="cps")
            nc.tensor.matmul(cps[:], ONES[:], acc[:], start=True, stop=True)
            nc.vector.tensor_copy(
                OUTCS[0:1, slot * BLK:(slot + 1) * BLK], cps[0:1, :]
            )

        def cs_adds(ets, n):
            """pairwise-tree DVE adds of the exp tiles -> one bf16 tile."""
            if n == 2:
                x = apool.tile([P, BLK], _BF16, tag="acc")
                nc.vector.tensor_add(x[:], ets[0][:], ets[1][:])
                return x
            x01 = apool.tile([P, BLK], _BF16, tag="acc")
            nc.vector.tensor_add(x01[:], ets[0][:], ets[1][:])
            x23 = apool.tile([P, BLK], _BF16, tag="acc")
            nc.vector.tensor_add(x23[:], ets[2][:], ets[3][:])
            xs = apool.tile([P, BLK], _BF16, tag="acc")
            nc.vector.tensor_add(xs[:], x01[:], x23[:])
            return xs

        def mm(ps, k, m, xsrc, xoff, wsrc=None, woff=0):
            nc.tensor.matmul(
                ps[:],
                (wsrc or A)[k][:, :, woff + m * P: woff + (m + 1) * P],
                xsrc[k][:, :, xoff: xoff + BLK],
                start=(k == 0),
                stop=(k == KT - 1),
                perf_mode=_DR,
            )

        # --- phase 1: k-chase the (A[k], B2[k]) arrivals ---
        ps1 = [psum.tile([P, BLK], _F32, tag="ps", name=f"ps1_{m}")
               for m in range(MT)]
        ps2 = [psum.tile([P, BLK], _F32, tag="ps", name=f"ps2_{m}")
               for m in range(2)]
        for k in range(KT):
            for m in range(MT):
                mm(ps1[m], k, m, A, BLK)
            for m in range(2):
                mm(ps2[m], k, m, B2, 0)
            if k < KT - 1:  # HAM keep-alive if the next pair is late
                nc.tensor.matmul(wps[:], ZW[:, 0:P], ZW[:], start=True, stop=True)

        ets1 = [act_rs(ps1[m], RS_B1 + m) for m in range(MT)]
        csa1 = cs_adds(ets1, 4)

        # CE while ScalarE has slack: R[p, t] = sum_c exp(y_pred)
        for t in range(MT):
            etc = cepool.tile([P, C], _F32, tag="etce")
            nc.scalar.activation(
                etc[:], YPB[:, t * C:(t + 1) * C], _EXP,
                bias=bias_z[:], scale=1.0,
                accum_out=OUTSB[:, RS_CE + t:RS_CE + t + 1],
            )

        # --- phase 2 ---
        # b2 m2/m3 (m-outer), ACTs for b2 m0..3
        ets2 = [None] * MT
        psb = []
        for m in (2, 3):
            ps = psum.tile([P, BLK], _F32, tag="ps")
            for k in range(KT):
                mm(ps, k, m, B2, 0)
            psb.append(ps)
        ets2[0] = act_rs(ps2[0], RS_B2 + 0)
        ets2[1] = act_rs(ps2[1], RS_B2 + 1)
        ets2[2] = act_rs(psb[0], RS_B2 + 2)
        ets2[3] = act_rs(psb[1], RS_B2 + 3)
        csa2 = cs_adds(ets2, 4)

        # b3 (X = c3 at BR offset 0); onesMM(b1) after m0, (b2) after m2
        ets3 = []
        for m in range(MT):
            ps = psum.tile([P, BLK], _F32, tag="ps")
            for k in range(KT):
                mm(ps, k, m, BR, 0)
            ets3.append(act_rs(ps, RS_B3 + m))
            if m == 0:
                ones_mm(csa1, CS_B1)
            if m == 2:
                ones_mm(csa2, CS_B2)
        csa3 = cs_adds(ets3, 4)

        # b4 (half: W = wh at BR 2*BLK, X = c4 at BR BLK)
        ets4 = []
        for m in range(2):
            ps = psum.tile([P, BLK], _F32, tag="ps")
            for k in range(KT):
                mm(ps, k, m, BR, BLK, wsrc=BR, woff=2 * BLK)
            ets4.append(act_rs(ps, RS_B4 + m))
        csa4 = cs_adds(ets4, 2)

        # b0 (diag, X = c0 at A offset 0): RS only, short tail
        for m in range(MT):
            ps = psum.tile([P, BLK], _F32, tag="ps")
            for k in range(KT):
                mm(ps, k, m, A, 0)
            act_rs(ps, RS_B0 + m)
            if m == 0:
                ones_mm(csa3, CS_B3)
            if m == 1:
                ones_mm(csa4, CS_B4)
            if m == 2:
                nc.gpsimd.dma_start(out_cs[:], OUTCS[:])

        nc.scalar.dma_start(out_rs[:], OUTSB[:])


_NC_CACHE = None


def _get_nc():
    global _NC_CACHE
    if _NC_CACHE is None:
        nc = bacc.Bacc(
            "TRN2", target_bir_lowering=False, debug=False,
            enable_asserts=False, num_devices=NCORES,
        )
        biga_d = nc.dram_tensor("biga", [KT * P, 2 * AW], _FP8, kind="ExternalInput")
        bigb2_d = nc.dram_tensor("bigb2", [KT * P, 2 * BLK], _FP8, kind="ExternalInput")
        bigbr_d = nc.dram_tensor("bigbr", [KT * P, 2 * RW], _FP8, kind="ExternalInput")
        yp_d = nc.dram_tensor("yp", [P, MT * C], _BF16, kind="ExternalInput")
        out_rs_d = nc.dram_tensor("out_rs", [P, NRS], _F32, kind="ExternalOutput")
        out_cs_d = nc.dram_tensor("out_cs", [1, 4 * BLK], _F32, kind="ExternalOutput")
        with tile.TileContext(nc) as tc:
            _build_kernel(
                tc, biga_d.ap(), bigb2_d.ap(), bigbr_d.ap(), yp_d.ap(),
                out_rs_d.ap(), out_cs_d.ap(),
            )
        nc.compile()
        _NC_CACHE = nc
    return _NC_CACHE


def _pack_cols(zq, cols_list):
    """[D, *] fp8 col-chunks -> [KT*P, 2*W] with the DoubleRow pairing
    row kk*128+p, col i*W+n  <->  contraction index kk*256 + 128i + p."""
    cat = np.concatenate(cols_list, axis=1)  # [D, W]
    w = cat.shape[1]
    return np.ascontiguousarray(
        cat.reshape(KT, 2, P, w).transpose(0, 2, 1, 3).reshape(KT * P, 2 * w)
    )


def _run_device(xnT, y_pred, trace=False):
    """Run the SPMD kernel; returns (T[N], R[N]) f64 and the raw results."""
    fp8np = mybir.dt.np(_FP8)
    bf16np = mybir.dt.np(_BF16)
    zq = (xnT * S8).astype(np.float32).astype(fp8np)  # [D, N] fp8
    ch = lambda j: zq[:, (j % 8) * BLK:(j % 8) * BLK + BLK]
    in_maps = []
    for c in range(NCORES):
        blk = slice(c * BLK, (c + 1) * BLK)
        ypb = (
            np.ascontiguousarray(y_pred[blk])
            .reshape(MT, P, C).transpose(1, 0, 2).reshape(P, MT * C)
        )
        if c < 4:
            c4 = ch(c + 4)
            wh = ch(c)[:, 0:256]
        else:
            c4 = ch(c)
            wh = ch(c - 4)[:, 256:512]
        in_maps.append({
            "biga": _pack_cols(zq, [ch(c), ch(c + 1)]),
            "bigb2": _pack_cols(zq, [ch(c + 2)]),
            "bigbr": _pack_cols(zq, [ch(c + 3), c4, wh]),
            "yp": np.ascontiguousarray(ypb).astype(bf16np),
        })
    res = run_bass_kernel_spmd(
        _get_nc(), in_maps, core_ids=list(range(NCORES)), trace=trace,
    )
    T = np.zeros(N, np.float64)
    R = np.empty(N, np.float64)
    for c, r in enumerate(res.results):
        o = r["out_rs"].astype(np.float64)            # [128, 22]
        cs = r["out_cs"].astype(np.float64).reshape(4, BLK)
        for m in range(MT):
            rows = slice(c * BLK + m * P, c * BLK + (m + 1) * P)
            T[rows] += (o[:, RS_B1 + m] + o[:, RS_B2 + m]
                        + o[:, RS_B3 + m] + o[:, RS_B0 + m])
            R[rows] = o[:, RS_CE + m]
        # half-block row sums
        if c < 4:
            p0 = c * BLK
            T[p0:p0 + P] += o[:, RS_B4]
            T[p0 + P:p0 + 2 * P] += o[:, RS_B4 + 1]
        else:
            p0 = (c - 4) * BLK
            T[p0 + 2 * P:p0 + 3 * P] += o[:, RS_B4]
            T[p0 + 3 * P:p0 + 4 * P] += o[:, RS_B4 + 1]
        # column sums
        chs = lambda j: slice((j % 8) * BLK, (j % 8) * BLK + BLK)
        T[chs(c + 1)] += cs[CS_B1]
        T[chs(c + 2)] += cs[CS_B2]
        T[chs(c + 3)] += cs[CS_B3]
        T[chs(c + 4 if c < 4 else c)] += cs[CS_B4]
    return T, R, res


def kernel(layer_embeds, y_true, y_pred):
    x = np.asarray(layer_embeds, dtype=np.float32)
    yt = np.asarray(y_true).astype(np.int64)
    yp = np.asarray(y_pred, dtype=np.float32)

    # normalize rows (torch-style eps clip)
    norms = np.maximum(
        np.sqrt((x.astype(np.float64) ** 2).sum(1, keepdims=True)), 1e-8
    )
    xn = (x / norms).astype(np.float32)
    xnT = np.ascontiguousarray(xn.T)  # [D, N]

    trace = bool(int(os.environ.get("CLCE_TRACE", "0")))
    T, R, res = _run_device(xnT, yp, trace=trace)
    if trace:
        kernel.last_results = res

    # --- host-side small terms (O(N * class_size)) ---
    # P_ must match what the device summed for the same-class entries, i.e.
    # the fp8-quantized sim values, so quantize the same way here.
    fp8np = mybir.dt.np(_FP8)
    xq = (xn * S8).astype(fp8np).astype(np.float64) / S8  # device-visible xn
    counts = np.bincount(yt, minlength=C)
    P_ = np.zeros(N, np.float64)
    slot0 = np.zeros(N, np.float64)
    for cval in np.unique(yt):
        idx = np.where(yt == cval)[0]
        subq = xq[idx]
        sq = (subq @ subq.T + 1.0) * (0.5 * TAU)   # device-matching sim
        P_[idx] = np.exp(sq).sum(1)
        if len(idx) >= 2:
            # slot0 feeds the final formula directly -> use full precision
            sub = xn[idx].astype(np.float64)
            s = (sub @ sub.T + 1.0) * (0.5 * TAU)
            firstpos = np.where(np.arange(len(idx)) == 0, 1, 0)
            slot0[idx] = s[np.arange(len(idx)), firstpos]

    num_neg = N - counts[yt]
    S = T - P_
    Z = (2 * N - 2 - num_neg).astype(np.float64)
    cl = (np.log(np.exp(slot0) + S + Z) - slot0).mean()
    ce = (
        np.log(R) - yp[np.arange(N), yt].astype(np.float64)
    ).mean()
    loss = LAMBD * cl + (1.0 - LAMBD) * ce
    return np.asarray(loss, dtype=np.float32)
